# revision 1
# baseline (speedup 1.0000x reference)
"""Trainium2 Bass kernel for the ICP depth-term loss (bidirectional masked
nearest-neighbour correspondence + mean distance).

Semantics (validated vs reference to ~7e-8 in fp64): for each query q against
refs {r}, dv = min d2 over cos-valid refs, dmin = min d2 over all refs,
picked = dv if dv < TH2 else dmin, loss = mean(sqrt(picked)) summed over both
directions.  The masked min uses max(d2, B*(COS-cos)), which equals d2 for
cos-valid pairs and is huge otherwise - no relu pass needed.

Structure: d2 and cos grids are SYMMETRIC, so ONE pass over the
[depth x verts] grid serves BOTH directions.  Depth is sharded across the 8
cores (6272 points each).  Per (depth-tile dt, vert-half vst) the grid tile
is [128 depth (partitions), 3584 verts (free)]:

  PE   psD = d2 (K=15 hi/lo features, 7 x 512-col matmuls)
  ACT  d2b = bf16(psD) -> right half of a [128,7168] cat tile
  PE   psC = B*(COS-cos) (K=11), reusing the same 7 PSUM banks
  DVE  cat left half = max(d2b, psC)            (masked values)
  DVE  runA[vst] = min(runA[vst], cat)          (direction A, both quantities)
  DVE  2x tensor_reduce(cat halves) -> outB     (direction B, per depth point)

This instance's execution cost is dominated by a fixed per-instruction
overhead, so the kernel maximizes per-instruction work: 3584-wide ACT/DVE
ops, no second-level device reductions, and the direction-A partition-axis
collapse + threshold select + sqrt + mean epilogue (O(N+M)) done on host
from the DMA'd running-min tiles.
"""

import math

import numpy as np

import concourse.bacc as bacc
import concourse.tile as tile
from concourse import mybir
from concourse.bass_utils import run_bass_kernel_spmd
from concourse.tile_rust import add_dep_helper

N_VERTS = 6890
M_DEPTH = 50000
N_CORES = 8

DQ = 6272            # depth points per core (49 tiles x 128 partitions)
NDT = 49
RV = 7168            # verts padded
FD = 3584            # verts per grid tile (7 PSUM banks)
NVST = RV // FD      # 2

TH2 = 0.05 * 0.05
COS = math.cos(math.pi / 12.0)
B = 1.0e4            # cos-penalty scale: pen = B*(COS - cos)
PAD = 60.0           # padding coordinate: d2 >= 10000 vs any real point

F32 = mybir.dt.float32
BF16 = mybir.dt.bfloat16
AX = mybir.AxisListType.X
OP = mybir.AluOpType


def _build(repeat=1):
    nc = bacc.Bacc("TRN2")

    depD = nc.declare_dram_parameter("depD", [15, DQ], BF16, isOutput=False)
    depC = nc.declare_dram_parameter("depC", [11, DQ], BF16, isOutput=False)
    vertD = nc.declare_dram_parameter("vertD", [15, RV], BF16, isOutput=False)
    vertC = nc.declare_dram_parameter("vertC", [11, RV], BF16, isOutput=False)
    outR_d = nc.declare_dram_parameter("outR", [128, 2 * RV], BF16,
                                       isOutput=True)
    outB_d = nc.declare_dram_parameter("outB", [128, 4 * NDT + 1], F32,
                                       isOutput=True)

    from contextlib import ExitStack
    with ExitStack() as ctx:
        tc = ctx.enter_context(tile.TileContext(nc))
        singles = ctx.enter_context(tc.tile_pool(name="singles", bufs=1))
        work = ctx.enter_context(tc.tile_pool(name="work", bufs=2))
        ps_pool = ctx.enter_context(tc.tile_pool(name="ps", bufs=1,
                                                 space="PSUM"))
        psdum = ctx.enter_context(tc.tile_pool(name="psdum", bufs=1,
                                               space="PSUM"))

        depD_sb = singles.tile([15, DQ], BF16)
        depC_sb = singles.tile([11, DQ], BF16)
        vertD_sb = singles.tile([15, RV], BF16)
        vertC_sb = singles.tile([11, RV], BF16)
        nc.gpsimd.dma_start(out=depD_sb, in_=depD[:, :])
        nc.gpsimd.dma_start(out=depC_sb, in_=depC[:, :])
        nc.gpsimd.dma_start(out=vertD_sb, in_=vertD[:, :])
        nc.gpsimd.dma_start(out=vertC_sb, in_=vertC[:, :])

        outB_sb = singles.tile([128, 4 * NDT + 1], F32)
        runA = [singles.tile([128, 2 * FD], BF16, tag=f"runA{i}",
                             name=f"runA{i}")
                for i in range(NVST)]

        # One tiny PE matmul per resident DMA so the PE observes each DMA's
        # semaphore once; later matmuls ride on PE program order (a Matmult
        # carries at most one semaphore wait).
        dumT = psdum.tile([1, 1], F32, tag="dum")
        observers = []
        for sb in (depD_sb, depC_sb, vertD_sb, vertC_sb):
            observers.append(
                nc.tensor.matmul(dumT, sb[:, 0:1], sb[:, 0:1],
                                 start=True, stop=True, skip_group_check=True))
        nc.vector.memset(outB_sb[:, 4 * NDT:4 * NDT + 1], 0.0)
        nc.vector.tensor_copy(outB_sb[0:1, 4 * NDT:4 * NDT + 1], dumT)

        for _rep in range(repeat):
            for dt in range(NDT):
                ds = slice(dt * 128, (dt + 1) * 128)
                for vst in range(NVST):
                    vbase = vst * FD
                    cat = work.tile([128, 2 * FD], BF16, tag="cat")
                    d2b = cat[:, FD:2 * FD]
                    mview = cat[:, 0:FD]

                    psD = ps_pool.tile([128, FD], F32, tag="ps")
                    mmD0 = None
                    for h in range(0, FD, 512):
                        mm = nc.tensor.matmul(
                            psD[:, h:h + 512], depD_sb[:, ds],
                            vertD_sb[:, vbase + h:vbase + h + 512],
                            start=True, stop=True)
                        mmD0 = mmD0 if mmD0 is not None else mm
                    if _rep == 0 and dt == 0 and vst == 0:
                        for obs in observers:
                            add_dep_helper(mmD0.ins, obs.ins, sync=False,
                                           reason="observe DMA before matmul")
                    nc.scalar.activation(d2b, psD,
                                         mybir.ActivationFunctionType.Copy)

                    psC = ps_pool.tile([128, FD], F32, tag="ps")
                    for h in range(0, FD, 512):
                        nc.tensor.matmul(
                            psC[:, h:h + 512], depC_sb[:, ds],
                            vertC_sb[:, vbase + h:vbase + h + 512],
                            start=True, stop=True)
                    # masked values: m = max(d2, pen); pen <= 0 iff cos-valid
                    nc.vector.tensor_tensor(mview, d2b, psC, op=OP.max)

                    if dt == 0:
                        nc.vector.tensor_copy(runA[vst], cat)
                    else:
                        nc.vector.tensor_tensor(runA[vst], runA[vst], cat,
                                                op=OP.min)
                    # direction-B minima for this (dt, vst)
                    c0 = dt * 4 + vst * 2
                    nc.vector.tensor_reduce(out=outB_sb[:, c0:c0 + 1],
                                            in_=mview, axis=AX, op=OP.min)
                    nc.vector.tensor_reduce(out=outB_sb[:, c0 + 1:c0 + 2],
                                            in_=d2b, axis=AX, op=OP.min)

        for vst in range(NVST):
            nc.gpsimd.dma_start(
                out=outR_d[:, vst * 2 * FD:(vst + 1) * 2 * FD],
                in_=runA[vst])
        nc.gpsimd.dma_start(out=outB_d[:, :], in_=outB_sb)

    nc.finalize()
    _dedup_ldweights(nc)
    return nc


def _dedup_ldweights(nc):
    """Drop InstLdweights whose stationary AP is identical to the weights
    already loaded in the PE array.  bass emits one ldweights per matmul;
    runs of matmuls that share a stationary tile (the 7 x 512-col column
    tiles of one grid row) only need the first.  Weights persist in the PE
    across other engines' instructions; anything unexpected resets tracking.
    Sync lives on separate event-semaphore instructions, and the first
    (kept) load of each run is the one carrying the run's dependencies."""
    import os
    if os.environ.get("BASS_NO_DEDUP"):
        return
    def ap_key(a):
        return (str(a.ap), a.offset, str(a.dtype))
    for f in nc.m.functions:
        for blk in f.blocks:
            out = []
            last = None
            for inst in blk.instructions:
                n = inst.__class__.__name__
                if n == "InstLdweights":
                    key = (ap_key(inst.ins[0]),
                           str(getattr(inst, "perf_mode", None)),
                           str(getattr(inst, "is_transpose", None)))
                    if key == last:
                        continue
                    last = key
                elif n == "InstMatmult":
                    pass  # same weights remain loaded
                elif n in ("InstEventSemaphore", "InstDrain", "InstDMACopy",
                           "InstActivation", "InstTensorTensor",
                           "InstTensorReduce", "InstTensorCopy", "InstMemset",
                           "InstTensorScalarPtr", "InstLoadActFuncSet"):
                    pass  # non-PE / sync instructions don't touch PE weights
                else:
                    last = None
                out.append(inst)
            blk.instructions = out


def _pack_inputs(depth_vmap, depth_nmap, verts_src, normal_src):
    import ml_dtypes
    BF = ml_dtypes.bfloat16

    d = np.ascontiguousarray(np.asarray(depth_vmap, dtype=np.float32))
    nd = np.ascontiguousarray(np.asarray(depth_nmap, dtype=np.float32))
    v = np.ascontiguousarray(np.asarray(verts_src, dtype=np.float32))
    nv = np.ascontiguousarray(np.asarray(normal_src, dtype=np.float32))

    def split(x):
        hi = x.astype(BF).astype(np.float32)
        lo = (x - hi).astype(BF).astype(np.float32)
        return hi, lo

    dep = np.full((N_CORES * DQ, 3), PAD, np.float32); dep[:M_DEPTH] = d
    depn = np.zeros((N_CORES * DQ, 3), np.float32); depn[:, 0] = 1.0
    depn[:M_DEPTH] = nd
    vert = np.full((RV, 3), PAD, np.float32); vert[:N_VERTS] = v
    vertn = np.zeros((RV, 3), np.float32); vertn[:, 0] = 1.0
    vertn[:N_VERTS] = nv

    # d2 features: K=15 hi/lo split, d2 = |q|^2 + |r|^2 - 2 q.r
    q = dep.T; qh, ql = split(q)
    q2 = (dep.astype(np.float64) ** 2).sum(1).astype(np.float32)
    q2h, q2l = split(q2)
    depDv = np.zeros((15, N_CORES * DQ), np.float32)
    depDv[0:3] = qh; depDv[3] = q2h; depDv[4] = 1.0
    depDv[5:8] = ql; depDv[8] = q2l; depDv[9] = 0.0
    depDv[10:13] = qh; depDv[13] = 0.0; depDv[14] = 1.0

    t = -2.0 * vert.T; th, tl = split(t)
    r2 = (vert.astype(np.float64) ** 2).sum(1).astype(np.float32)
    r2h, r2l = split(r2)
    vertDv = np.zeros((15, RV), np.float32)
    vertDv[0:3] = th; vertDv[3] = 1.0; vertDv[4] = r2h
    vertDv[5:8] = th; vertDv[8] = 1.0; vertDv[9] = r2h
    vertDv[10:13] = tl; vertDv[13] = 0.0; vertDv[14] = r2l

    # cos-penalty features: pen = B*COS - B*(nd.nv), hi/lo split products
    ndh, ndl = split(depn.T)
    depCv = np.zeros((11, N_CORES * DQ), np.float32)
    depCv[0:3] = ndh; depCv[3:6] = ndh; depCv[6:9] = ndl
    depCv[9] = 1.0; depCv[10] = 1.0
    nvh, nvl = split(vertn.T)
    bias = np.float32(B * COS)
    bh = np.float32(BF(bias)); bl = np.float32(BF(np.float32(bias - bh)))
    vertCv = np.zeros((11, RV), np.float32)
    vertCv[0:3] = -B * nvh; vertCv[3:6] = -B * nvl; vertCv[6:9] = -B * nvh
    vertCv[9] = bh; vertCv[10] = bl

    vertD_bf = vertDv.astype(BF); vertC_bf = vertCv.astype(BF)
    depD_bf = depDv.astype(BF); depC_bf = depCv.astype(BF)

    in_maps = []
    for c in range(N_CORES):
        cs = slice(c * DQ, (c + 1) * DQ)
        in_maps.append({
            "depD": np.ascontiguousarray(depD_bf[:, cs]),
            "depC": np.ascontiguousarray(depC_bf[:, cs]),
            "vertD": vertD_bf,
            "vertC": vertC_bf,
        })
    return in_maps


_CACHE = {}


def _cache_nc():
    if "nc" not in _CACHE:
        _CACHE["nc"] = _build()
    return _CACHE["nc"]


def kernel(depth_vmap, depth_nmap, verts_src, normal_src, k, _cache=_CACHE):
    in_maps = _pack_inputs(depth_vmap, depth_nmap, verts_src, normal_src)
    res = run_bass_kernel_spmd(_cache_nc(), in_maps,
                               core_ids=list(range(N_CORES)))

    allV = np.empty((N_CORES, RV), np.float32)
    allM = np.empty((N_CORES, RV), np.float32)
    dvB = np.empty(N_CORES * DQ, np.float32)
    dmB = np.empty(N_CORES * DQ, np.float32)
    for c, r in enumerate(res.results):
        outR = np.asarray(r["outR"]).astype(np.float32)   # [128, 2*RV]
        outB = r["outB"]
        for vst in range(NVST):
            base = vst * 2 * FD
            vs = slice(vst * FD, (vst + 1) * FD)
            allV[c, vs] = outR[:, base:base + FD].min(0)
            allM[c, vs] = outR[:, base + FD:base + 2 * FD].min(0)
        dv = np.minimum(outB[:, 0:4 * NDT:4], outB[:, 2:4 * NDT:4])  # [128,49]
        dm = np.minimum(outB[:, 1:4 * NDT:4], outB[:, 3:4 * NDT:4])
        dvB[c * DQ:(c + 1) * DQ] = dv.T.reshape(DQ)
        dmB[c * DQ:(c + 1) * DQ] = dm.T.reshape(DQ)

    dvA = allV.min(0)[:N_VERTS]; dmA = allM.min(0)[:N_VERTS]
    pickA = np.where(dvA < TH2, dvA, dmA)
    lossA = np.sqrt(np.maximum(pickA, 0, dtype=np.float64)).mean()
    dvB = dvB[:M_DEPTH]; dmB = dmB[:M_DEPTH]
    pickB = np.where(dvB < TH2, dvB, dmB)
    lossB = np.sqrt(np.maximum(pickB, 0, dtype=np.float64)).mean()
    return np.float32(lossA + lossB)


if __name__ == "__main__":
    rng = np.random.default_rng(0)
    d = rng.standard_normal((M_DEPTH, 3)).astype(np.float32)
    nd = rng.standard_normal((M_DEPTH, 3)).astype(np.float32)
    nd /= np.linalg.norm(nd, axis=1, keepdims=True)
    v = rng.standard_normal((N_VERTS, 3)).astype(np.float32)
    nv = rng.standard_normal((N_VERTS, 3)).astype(np.float32)
    print(kernel(d, nd, v, nv, 32))

